# revision 1
# baseline (speedup 1.0000x reference)
"""Trainium2 Bass kernel for nn_MultiHeadAttention_26259430048704.

Multi-head attention with additive bias and a multiplicative "explored" mask
applied to the scores before softmax (masked scores are set to 0, so they
contribute exp(0)=1 to the softmax).

Sharding: 16 heads / 8 cores = 2 heads per core (tensor parallel over heads).
Each core computes projections for its 128 W-columns and full attention for
its 2 heads; the host concatenates the per-core [128, 2048] transposed
outputs. No collectives.

Per-core dataflow (all matmuls bf16 with f32 PSUM accumulation):
  qhT/khT/vhT [128, N] = W.T @ x.T (+bias via per-partition add)
  scores computed transposed: s^T[m,n] = kh @ qh^T (K=64, both heads
  row-packed in the PE array), bias^T accumulated via identity matmul,
  exp on ACT, masked entries overwritten with 1.0 (DVE copy_predicated),
  p@V with vh' stationary ([m,65]; col 64 = ones so the softmax
  denominator Z falls out of the same accumulation), then out = num/Z.
"""

import sys

for _p in ("/opt/trn_rl_repo", "/root/.axon_site/_ro/trn_rl_repo"):
    if _p not in sys.path:
        sys.path.insert(0, _p)

import numpy as np
import ml_dtypes

BF16 = ml_dtypes.bfloat16

N = 2048
HID = 1024
HEADS = 16
DK = 64
NCORES = 8
HPC = HEADS // NCORES  # 2 heads per core
DC = HPC * DK  # 128 output columns per core
KT = HID // 128  # 8 contraction tiles
MT = N // 128  # 16 m tiles
NCH = N // 512  # 4 n chunks

_cache = {}

# tuning knobs (overridable before get_compiled)
CFG = {
    "ppsum_bufs": 2,
    "opsum_bufs": 2,
    "split_scores": True,  # per-head [128,512] scores psum instead of [128,1024]
    "scores_bufs": 4,
    "raw_on_act": False,
    "bias_fp8": True,
}


def _build(repeat=1):
    import concourse.bass as bass
    import concourse.bacc as bacc
    import concourse.mybir as mybir
    import concourse.tile as tile
    from concourse.masks import make_identity

    f32 = mybir.dt.float32
    bf16 = mybir.dt.bfloat16
    AF = mybir.ActivationFunctionType

    nc = bacc.Bacc("TRN2", target_bir_lowering=False, debug=False)

    xts = {t: nc.dram_tensor(f"xT{t}", [HID, N], bf16, kind="ExternalInput") for t in "qkv"}
    Ws = {t: nc.dram_tensor(f"W{t}", [128, KT * DC], bf16, kind="ExternalInput") for t in "qkv"}
    bs = {t: nc.dram_tensor(f"b{t}", [DC, 1], f32, kind="ExternalInput") for t in "qkv"}
    bias_dt = mybir.dt.float8e4 if CFG["bias_fp8"] else bf16
    biasT = nc.dram_tensor("biasT", [HPC * N, N], bias_dt, kind="ExternalInput")
    invm = nc.dram_tensor("invmaskT", [N, N], mybir.dt.uint8, kind="ExternalInput")
    outT = nc.dram_tensor("outT", [DC, N], f32, kind="ExternalOutput")

    with tile.TileContext(nc) as tc:
        with (
            tc.tile_pool(name="constp", bufs=1) as constp,
            tc.tile_pool(name="xtp", bufs=2) as xtp,
            tc.tile_pool(name="pers", bufs=1) as pers,
            tc.tile_pool(name="maskp", bufs=1) as maskp,
            tc.tile_pool(name="biasp", bufs=10) as biasp,
            tc.tile_pool(name="ep", bufs=6) as ep,
            tc.tile_pool(name="normp", bufs=4) as normp,
            tc.tile_pool(name="outp", bufs=4) as outp,
            tc.tile_pool(name="ppsum", bufs=CFG["ppsum_bufs"], space="PSUM") as ppsum,
            tc.tile_pool(name="spsum", bufs=(CFG["scores_bufs"] if CFG["split_scores"] else 2), space="PSUM") as spsum,
            tc.tile_pool(name="opsum", bufs=CFG["opsum_bufs"], space="PSUM") as opsum,
        ):
            ident = constp.tile([128, 128], bf16)
            make_identity(nc, ident)
            ident_b = ident
            if CFG["bias_fp8"]:
                ident_b = constp.tile([128, 128], mybir.dt.float8e4, name="ident_f8")
                make_identity(nc, ident_b)
            ones_bf = constp.tile([128, 512], bf16)
            nc.vector.memset(ones_bf, 1.0)
            ones_f32 = constp.tile([128, DK], f32)
            nc.vector.memset(ones_f32, 1.0)

            W_sb = {}
            b_sb = {}
            for t in "kvq":
                W_sb[t] = constp.tile([128, KT, DC], bf16, tag=f"w{t}", name=f"W{t}_sb")
                nc.sync.dma_start(
                    out=W_sb[t], in_=Ws[t].ap().rearrange("p (kt m) -> p kt m", kt=KT)
                )
                b_sb[t] = constp.tile([DC, 1], f32, tag=f"b{t}", name=f"b{t}_sb")
                nc.sync.dma_start(out=b_sb[t], in_=bs[t].ap())

            maskt = maskp.tile([128, MT, N], mybir.dt.uint8)

            def emit_body():
                # ---- Phase 1: projections ----
                proj = {}
                for t in "kvq":
                    xt_sb = xtp.tile([128, KT, N], bf16, tag="xt", name=f"xt_{t}")
                    xt_dram = xts[t].ap().rearrange("(kt p) n -> p kt n", p=128)
                    for kt in range(KT):
                        for ch2 in range(2):
                            nc.sync.dma_start(
                                out=xt_sb[:, kt, ch2 * 1024 : (ch2 + 1) * 1024],
                                in_=xt_dram[:, kt, ch2 * 1024 : (ch2 + 1) * 1024],
                            )
                    proj[t] = pers.tile([128, N], bf16, tag=f"proj{t}", name=f"proj{t}_sb")
                    for ch in range(NCH):
                        ps = ppsum.tile([128, 512], f32, tag="pp", name="proj_ps")
                        for kt in range(KT):
                            nc.tensor.matmul(
                                ps,
                                lhsT=W_sb[t][:, kt, :],
                                rhs=xt_sb[:, kt, ch * 512 : (ch + 1) * 512],
                                start=(kt == 0),
                                stop=(kt == KT - 1),
                            )
                        # psum + per-partition bias -> bf16 (DVE)
                        nc.vector.tensor_scalar_add(
                            proj[t][:, ch * 512 : (ch + 1) * 512], ps, b_sb[t]
                        )

                invm_r = invm.ap().rearrange("(mt p) n -> p mt n", p=128)
                for _mt in range(MT):
                    nc.sync.dma_start(out=maskt[:, _mt, :], in_=invm_r[:, _mt, :])

                # vh' per head: [m-part, mt, 65]; col 64 = ones (softmax denom)
                vhp = [
                    pers.tile([128, MT, DK + 1], bf16, tag=f"vhp{h}", name=f"vhp{h}_sb")
                    for h in range(HPC)
                ]
                for h in range(HPC):
                    nc.vector.memset(vhp[h][:, :, DK : DK + 1], 1.0)
                for mb in range(MT):
                    pstr = ppsum.tile([128, 128], bf16, tag="pp", name="tr_ps")
                    nc.tensor.transpose(
                        pstr, proj["v"][:, mb * 128 : (mb + 1) * 128], ident
                    )
                    for h in range(HPC):
                        nc.vector.tensor_copy(
                            vhp[h][:, mb, 0:DK], pstr[:, h * DK : (h + 1) * DK]
                        )

                # ---- Phase 2: attention ----
                # Flat software pipeline over (nch, mt): the pV matmul for a
                # tile is emitted one step late so the PE never head-of-line
                # blocks waiting for that tile's exp/mask; pout accumulators
                # are allocated at first use and drained right after their
                # final pV, letting the PE run across n-chunk boundaries.
                state = {"pouts": None}
                pending = []

                def emit_pv(item):
                    e3, m, n0_, pouts_ = item
                    for h in range(HPC):
                        nc.tensor.matmul(
                            pouts_[h][0 : DK + 1, :],
                            lhsT=vhp[h][:, m, :],
                            rhs=e3[:, h, :],
                            start=(m == 0),
                            stop=(m == MT - 1),
                        )
                    if m == MT - 1:
                        emit_norm(n0_, pouts_)

                def emit_norm(n0_, pouts_):
                    for h in range(HPC):
                        raw = normp.tile([128, 512], f32, tag="raw", name="raw_t")
                        if CFG["raw_on_act"]:
                            nc.scalar.copy(raw[0 : DK + 1, :], pouts_[h][0 : DK + 1, :])
                        else:
                            nc.vector.tensor_copy(
                                raw[0 : DK + 1, :], pouts_[h][0 : DK + 1, :]
                            )
                        lnz = normp.tile([128, 512], f32, tag="lnz", name="lnz_t")
                        nc.scalar.activation(
                            lnz[DK : DK + 1, :], raw[DK : DK + 1, :], AF.Ln
                        )
                        rz = normp.tile([128, 512], f32, tag="rz", name="rz_t")
                        nc.scalar.activation(
                            rz[DK : DK + 1, :], lnz[DK : DK + 1, :], AF.Exp, scale=-1.0
                        )
                        # replicate 1/Z across partitions 0-63 via PE (K=1 matmul)
                        rzp = ppsum.tile([128, 512], f32, tag="pp", name="rz_ps")
                        nc.tensor.matmul(
                            rzp[0:DK, :],
                            lhsT=ones_f32[DK : DK + 1, :],
                            rhs=rz[DK : DK + 1, :],
                            start=True,
                            stop=True,
                        )
                        ot = outp.tile([128, 512], f32, tag="ot", name="ot_t")
                        nc.vector.tensor_mul(ot[0:DK, :], raw[0:DK, :], rzp[0:DK, :])
                        nc.sync.dma_start(
                            out=outT.ap()[h * DK : (h + 1) * DK, n0_ : n0_ + 512],
                            in_=ot[0:DK, :],
                        )

                for nch in range(NCH):
                    n0 = nch * 512
                    for mt in range(MT):
                        if mt == 0:
                            state["pouts"] = [
                                opsum.tile(
                                    [128, 512], f32, tag="po", name=f"pout{nch}_{h}"
                                )
                                for h in range(HPC)
                            ]
                        pouts = state["pouts"]
                        bt = biasp.tile([128, HPC, 512], bias_dt, tag="bt", name="bias_t")
                        bta = biasT.ap()
                        nc.sync.dma_start(
                            out=bt,
                            in_=bass.AP(
                                tensor=bta.tensor,
                                offset=bta.offset + mt * 128 * N + n0,
                                ap=[[N, 128], [N * N, HPC], [1, 512]],
                            ),
                        )
                        if CFG["split_scores"]:
                            pss = [
                                spsum.tile([128, 512], f32, tag="ps", name=f"score_ps{h}")
                                for h in range(HPC)
                            ]
                        else:
                            ps = spsum.tile([128, 1024], f32, tag="ps", name="score_ps")
                            pss = [ps[:, h * 512 : (h + 1) * 512] for h in range(HPC)]
                        # scores^T: kh @ qh^T (K=64; h0 rows 0-63, h1 rows 64-127)
                        for h in range(HPC):
                            nc.tensor.matmul(
                                pss[h],
                                lhsT=proj["k"][
                                    h * DK : (h + 1) * DK, mt * 128 : (mt + 1) * 128
                                ],
                                rhs=proj["q"][h * DK : (h + 1) * DK, n0 : n0 + 512],
                                start=True,
                                stop=False,
                            )
                        # += bias^T via identity matmul
                        for h in range(HPC):
                            nc.tensor.matmul(
                                pss[h],
                                lhsT=ident_b,
                                rhs=bt[:, h, :],
                                start=False,
                                stop=True,
                            )
                        et = ep.tile([128, 1024], bf16, tag="et", name="e_t")
                        et3 = et.rearrange("p (h n) -> p h n", h=HPC)
                        if CFG["split_scores"]:
                            for h in range(HPC):
                                nc.scalar.activation(et3[:, h, :], pss[h], AF.Exp)
                        else:
                            nc.scalar.activation(et, ps, AF.Exp)
                        for h in range(HPC):
                            nc.vector.copy_predicated(
                                et3[:, h, :], maskt[:, mt, n0 : n0 + 512], ones_bf
                            )
                        pending.append((et3, mt, n0, pouts))
                        if len(pending) > 4:
                            emit_pv(pending.pop(0))
                while pending:
                    emit_pv(pending.pop(0))

            if repeat == 1:
                emit_body()
            else:
                with tc.For_i(
                    0,
                    repeat,
                    1,
                    hint_engines=(
                        mybir.EngineType.PE,
                        mybir.EngineType.DVE,
                        mybir.EngineType.Activation,
                        mybir.EngineType.SP,
                    ),
                ):
                    emit_body()

    nc.compile()
    return nc


def _wlayout(w):
    # [HID, DC] -> [128, KT*DC]: partition-major k-tile layout, contiguous DMA
    return np.ascontiguousarray(
        w.reshape(KT, 128, DC).transpose(1, 0, 2).reshape(128, KT * DC)
    ).astype(BF16)


def stage_inputs(q, k, v, attn_bias, explored, Wq, bq, Wk, bk, Wv, bv):
    """Host-side sharding/layout staging. Returns in_maps for 8 cores."""
    scale = DK ** -0.5
    xT = {
        "q": np.ascontiguousarray(np.asarray(q, np.float32).T).astype(BF16),
        "k": np.ascontiguousarray(np.asarray(k, np.float32).T).astype(BF16),
        "v": np.ascontiguousarray(np.asarray(v, np.float32).T).astype(BF16),
    }
    Wq = np.asarray(Wq, np.float32) * scale
    bq = np.asarray(bq, np.float32) * scale
    Wk = np.asarray(Wk, np.float32)
    bk = np.asarray(bk, np.float32)
    Wv = np.asarray(Wv, np.float32)
    bv = np.asarray(bv, np.float32)
    attn_bias = np.asarray(attn_bias, np.float32)
    explored = np.asarray(explored)

    # inverted keep-mask, transposed: 1 where score must be zeroed
    invmask = np.zeros((N, N), dtype=np.uint8)
    invmask[1:, 1:] = (explored == 0).T.astype(np.uint8)

    in_maps = []
    for c in range(NCORES):
        cols = slice(c * DC, (c + 1) * DC)
        h0 = HPC * c
        bdt = ml_dtypes.float8_e4m3 if CFG["bias_fp8"] else BF16
        bt = np.ascontiguousarray(
            attn_bias[h0 : h0 + HPC].transpose(0, 2, 1)
        ).astype(bdt).reshape(HPC * N, N)
        in_maps.append(
            {
                "xTq": xT["q"],
                "xTk": xT["k"],
                "xTv": xT["v"],
                "Wq": _wlayout(Wq[:, cols]),
                "Wk": _wlayout(Wk[:, cols]),
                "Wv": _wlayout(Wv[:, cols]),
                "bq": bq[cols].reshape(DC, 1).copy(),
                "bk": bk[cols].reshape(DC, 1).copy(),
                "bv": bv[cols].reshape(DC, 1).copy(),
                "biasT": bt,
                "invmaskT": invmask,
            }
        )
    return in_maps


def assemble_output(results):
    """results: list of 8 dicts with 'outT' [128, 2048] f32."""
    out = np.empty((N, HEADS * DK), dtype=np.float32)
    for c in range(NCORES):
        r = np.asarray(results[c]["outT"])
        for j in range(HPC):
            h = HPC * c + j
            out[:, h * DK : (h + 1) * DK] = r[j * DK : (j + 1) * DK, :].T
    return out


def get_compiled(repeat=1):
    key = ("nc", repeat, tuple(sorted(CFG.items())))
    if key not in _cache:
        _cache[key] = _build(repeat)
    return _cache[key]


def kernel(**inputs) -> np.ndarray:
    from concourse.bass_utils import run_bass_kernel_spmd

    nc = get_compiled()
    in_maps = stage_inputs(**inputs)
    res = run_bass_kernel_spmd(nc, in_maps, core_ids=list(range(NCORES)))
    return assemble_output(res.results)



# revision 19
# speedup vs baseline: 1.3584x; 1.3584x over previous
"""Trainium2 Bass kernel for nn_MultiHeadAttention_26259430048704.

Multi-head attention with additive bias and a multiplicative "explored" mask
applied to the scores before softmax (masked scores are set to 0, so they
contribute exp(0)=1 to the softmax).

Sharding: 16 heads / 8 cores = 2 heads per core (tensor parallel over heads).
Each core computes projections for its 128 W-columns and full attention for
its 2 heads; the host concatenates the per-core [2048, 128] outputs along the
feature axis. No collectives.

Per-core dataflow (matmuls bf16/fp8 with f32 PSUM accumulation):
  qhT/khT/vhT [128, N] = W.T @ x.T (+bias via per-partition add)
  scores^T[m,n] per head = kh @ qh^T (K=64) into one [128, 1024] psum,
  bias^T (fp8, with -256 folded in at masked entries) accumulated via
  identity matmul, exp on ACT (masked entries underflow to 0), then
  output-transposed pV: out[n, d] accumulates e-tile-stationary matmuls
  (65-col streams) PLUS a mask-correction matmul per m-tile
  (lhsT = invmask tile, rhs = [vh0|1|vh1|1]) that restores the masked
  entries' exp(0)=1 contributions to both numerator and Z. Normalization
  is a per-partition reciprocal + tensor_scalar multiply (Z rides along
  as the ones columns of the pV/correction rhs).
"""

import sys

for _p in ("/opt/trn_rl_repo", "/root/.axon_site/_ro/trn_rl_repo"):
    if _p not in sys.path:
        sys.path.insert(0, _p)

import numpy as np
import ml_dtypes

BF16 = ml_dtypes.bfloat16
F16 = np.float16
F8 = ml_dtypes.float8_e4m3

N = 2048
HID = 1024
HEADS = 16
DK = 64
NCORES = 8
HPC = HEADS // NCORES  # 2 heads per core
DC = HPC * DK  # 128 output columns per core
KT = HID // 128  # 8 contraction tiles
MT = N // 128  # 16 m tiles
NCH = N // 512  # 4 n chunks
VW = DK + 1  # 65: vh plus ones column (softmax denominator)

_cache = {}

# tuning knobs (overridable before get_compiled)
CFG = {
    "ppsum_bufs": 2,
    "spsum_bufs": 3,
    "bias_bufs": 16,
    "et_bufs": 8,
    "pv_lag": 3,
    "mask_neg": -224.0,  # exactly representable in fp8e4m3 (IEEE, max 240)
    # fraction of score tiles whose bias is injected via PE identity matmul;
    # the rest use a DVE tensor add (engine load balancing)
    "bias_pe_frac": 0.45,
    # python-level body unroll (for TimelineSim steady-state measurement)
    "py_unroll": 1,
}


def _build(repeat=1):
    import concourse.bass as bass
    import concourse.bacc as bacc
    import concourse.mybir as mybir
    import concourse.tile as tile
    from concourse.masks import make_identity

    f32 = mybir.dt.float32
    bf16 = mybir.dt.bfloat16
    f16 = mybir.dt.float16
    f8 = mybir.dt.float8e4
    AF = mybir.ActivationFunctionType

    nc = bacc.Bacc("TRN2", target_bir_lowering=False, debug=False)

    xts = {t: nc.dram_tensor(f"xT{t}", [HID, N], bf16, kind="ExternalInput") for t in "qkv"}
    Ws = {t: nc.dram_tensor(f"W{t}", [128, KT * DC], bf16, kind="ExternalInput") for t in "qkv"}
    bs = {t: nc.dram_tensor(f"b{t}", [DC, 1], f32, kind="ExternalInput") for t in "qkv"}
    biasT = nc.dram_tensor("biasT", [HPC * N, N], f8, kind="ExternalInput")
    invm = nc.dram_tensor("invmaskT", [N, N], f8, kind="ExternalInput")
    outD = nc.dram_tensor("outD", [N, DC], f16, kind="ExternalOutput")

    with tile.TileContext(nc) as tc:
        with (
            tc.tile_pool(name="constp", bufs=1) as constp,
            tc.tile_pool(name="xtp", bufs=2) as xtp,
            tc.tile_pool(name="pers", bufs=1) as pers,
            tc.tile_pool(name="maskp", bufs=1) as maskp,
            tc.tile_pool(name="biasp", bufs=CFG["bias_bufs"]) as biasp,
            tc.tile_pool(name="ep", bufs=CFG["et_bufs"]) as ep,
            tc.tile_pool(name="scp", bufs=4) as scp,
            tc.tile_pool(name="normp", bufs=4) as normp,
            tc.tile_pool(name="outp", bufs=2) as outp,
            # one shared PSUM pool: tag "ps" ring (proj/transpose/scores,
            # sized by the [128,1024] f32 scores tile = 2 banks each) plus
            # two single-buf pV accumulators (1 bank each)
            tc.tile_pool(name="psump", bufs=CFG["spsum_bufs"], space="PSUM") as psump,
        ):
            ident = constp.tile([128, 128], bf16)
            make_identity(nc, ident)
            ident_b = constp.tile([128, 128], f8, name="ident_f8")
            make_identity(nc, ident_b)

            W_sb = {}
            b_sb = {}
            for t in "kqv":
                W_sb[t] = constp.tile([128, KT, DC], bf16, tag=f"w{t}", name=f"W{t}_sb")
                nc.sync.dma_start(
                    out=W_sb[t], in_=Ws[t].ap().rearrange("p (kt m) -> p kt m", kt=KT)
                )
                b_sb[t] = constp.tile([DC, 1], f32, tag=f"b{t}", name=f"b{t}_sb")
                nc.sync.dma_start(out=b_sb[t], in_=bs[t].ap())

            maskt = maskp.tile([128, MT, N], f8)

            def emit_body():
                # ---- Phase 1: projections (k first: scores need k+q only) ----
                proj = {}
                for t in "kqv":
                    xt_sb = xtp.tile([128, KT, N], bf16, tag="xt", name=f"xt_{t}")
                    xt_dram = xts[t].ap().rearrange("(kt p) n -> p kt n", p=128)
                    # column-major order: all k-tiles of a column chunk land
                    # before the next chunk, so proj psum chunks start early
                    for ch2 in range(2):
                        for kt in range(KT):
                            nc.sync.dma_start(
                                out=xt_sb[:, kt, ch2 * 1024 : (ch2 + 1) * 1024],
                                in_=xt_dram[:, kt, ch2 * 1024 : (ch2 + 1) * 1024],
                            )
                    proj[t] = pers.tile([128, N], bf16, tag=f"proj{t}", name=f"proj{t}_sb")
                    for ch in range(NCH):
                        ps = psump.tile([128, 512], f32, tag="ps", name="proj_ps")
                        for kt in range(KT):
                            nc.tensor.matmul(
                                ps,
                                lhsT=W_sb[t][:, kt, :],
                                rhs=xt_sb[:, kt, ch * 512 : (ch + 1) * 512],
                                start=(kt == 0),
                                stop=(kt == KT - 1),
                            )
                        nc.vector.tensor_scalar_add(
                            proj[t][:, ch * 512 : (ch + 1) * 512], ps, b_sb[t]
                        )

                invm_r = invm.ap().rearrange("(mt p) n -> p mt n", p=128)
                for _mt in range(MT):
                    nc.sync.dma_start(out=maskt[:, _mt, :], in_=invm_r[:, _mt, :])

                # vh' both heads: [m-part, mt, 130]; cols 64/129 = ones
                vhp = pers.tile([128, MT, 2 * VW], f16, tag="vhp", name="vhp_sb")
                nc.vector.memset(vhp[:, :, DK : DK + 1], 1.0)
                nc.vector.memset(vhp[:, :, VW + DK : VW + DK + 1], 1.0)
                for mb in range(MT):
                    pstr = psump.tile([128, 128], bf16, tag="ps", name="tr_ps")
                    nc.tensor.transpose(
                        pstr, proj["v"][:, mb * 128 : (mb + 1) * 128], ident
                    )
                    for h in range(HPC):
                        nc.vector.tensor_copy(
                            vhp[:, mb, h * VW : h * VW + DK],
                            pstr[:, h * DK : (h + 1) * DK],
                        )

                # ---- Phase 2: attention ----
                # po tiles hold [num_h0 64 | Z_h0 | num_h1 64 | Z_h1] per n-row;
                # main pV (e-tile stationary, 65-col streams) and the mask
                # correction matmul accumulate into the same psum regions.
                state = {"po": None}
                pending = []

                def emit_pv(item):
                    et, m, po_ = item
                    for ns in range(4):
                        po = po_[ns // 2]
                        for h in range(HPC):
                            # start=True clears has_written for the whole
                            # bank; exactly one per po tile, all other
                            # writes overwrite-on-first-touch then accumulate
                            nc.tensor.matmul(
                                po[:, ns % 2, h * VW : (h + 1) * VW],
                                lhsT=et[:, h, ns * 128 : (ns + 1) * 128],
                                rhs=vhp[:, m, h * VW : (h + 1) * VW],
                                start=(m == 0 and ns % 2 == 0 and h == 0),
                                stop=False,
                                skip_group_check=True,
                            )
                    # mask correction: += invmask_tile.T @ [vh0|1|vh1|1]
                    n0_ = state["n0"]
                    for ns in range(4):
                        po = po_[ns // 2]
                        nc.tensor.matmul(
                            po[:, ns % 2, :],
                            lhsT=maskt[:, m, n0_ + ns * 128 : n0_ + (ns + 1) * 128],
                            rhs=vhp[:, m, :],
                            start=False,
                            stop=(m == MT - 1),
                            skip_group_check=True,
                        )
                    if m == MT - 1:
                        emit_norm(n0_, po_)

                def emit_norm(n0_, po_):
                    ot = outp.tile([128, 4, DC], f16, tag="ot", name="ot_t")
                    for ti in range(2):
                        po = po_[ti]
                        for h in range(HPC):
                            rz = normp.tile([128, 2, 1], f32, tag="rz", name="rz_t")
                            nc.vector.reciprocal(
                                rz, po[:, :, h * VW + DK : h * VW + DK + 1]
                            )
                            for k in range(2):
                                nc.vector.tensor_scalar_mul(
                                    ot[:, ti * 2 + k, h * DK : (h + 1) * DK],
                                    po[:, k, h * VW : h * VW + DK],
                                    rz[:, k, :],
                                )
                    nc.sync.dma_start(
                        out=outD.ap().rearrange(
                            "(nc ns p) d -> p nc ns d", p=128, ns=4
                        )[:, n0_ // 512],
                        in_=ot,
                    )

                bias_acc = 0.0
                for nch in range(NCH):
                    n0 = nch * 512
                    for mt in range(MT):
                        if mt == 0:
                            state["po"] = [
                                psump.tile(
                                    [128, 2, 2 * VW], f32, tag=f"po{i}",
                                    name=f"po{i}_t", bufs=1,
                                )
                                for i in range(2)
                            ]
                            state["n0"] = n0
                        bt = biasp.tile([128, HPC, 512], f8, tag="bt", name="bias_t")
                        bta = biasT.ap()
                        nc.sync.dma_start(
                            out=bt,
                            in_=bass.AP(
                                tensor=bta.tensor,
                                offset=bta.offset + mt * 128 * N + n0,
                                ap=[[N, 128], [N * N, HPC], [1, 512]],
                            ),
                        )
                        ps = psump.tile([128, HPC * 512], f32, tag="ps", name="score_ps")
                        ps3 = ps.rearrange("p (h n) -> p h n", h=HPC)
                        bias_acc += CFG["bias_pe_frac"]
                        bias_on_pe = bias_acc >= 1.0
                        if bias_on_pe:
                            bias_acc -= 1.0
                        # scores^T: kh @ qh^T (K=64)
                        for h in range(HPC):
                            nc.tensor.matmul(
                                ps3[:, h, :],
                                lhsT=proj["k"][
                                    h * DK : (h + 1) * DK, mt * 128 : (mt + 1) * 128
                                ],
                                rhs=proj["q"][h * DK : (h + 1) * DK, n0 : n0 + 512],
                                start=True,
                                stop=(not bias_on_pe),
                            )
                        # += bias^T (mask folded in as -224): identity matmul
                        # on PE for a fraction of tiles, DVE add otherwise
                        et = ep.tile([128, HPC, 512], f16, tag="et", name="e_t")
                        if bias_on_pe:
                            for h in range(HPC):
                                nc.tensor.matmul(
                                    ps3[:, h, :],
                                    lhsT=ident_b,
                                    rhs=bt[:, h, :],
                                    start=False,
                                    stop=True,
                                )
                            nc.scalar.activation(
                                et.rearrange("p h n -> p (h n)"), ps, AF.Exp
                            )
                        else:
                            # DVE adds bias to SBUF f16 (frees the psum slot
                            # early), then ACT exps from SBUF
                            sc = scp.tile([128, HPC, 512], f16, tag="sc", name="sc_t")
                            nc.vector.tensor_add(sc, ps3, bt)
                            nc.scalar.activation(
                                et.rearrange("p h n -> p (h n)"),
                                sc.rearrange("p h n -> p (h n)"),
                                AF.Exp,
                            )
                        pending.append((et, mt, state["po"]))
                        if len(pending) > CFG["pv_lag"]:
                            emit_pv(pending.pop(0))
                    while pending:
                        emit_pv(pending.pop(0))

            if repeat == 1:
                for _ in range(CFG["py_unroll"]):
                    emit_body()
            else:
                with tc.For_i(
                    0,
                    repeat,
                    1,
                    hint_engines=(
                        mybir.EngineType.PE,
                        mybir.EngineType.DVE,
                        mybir.EngineType.Activation,
                        mybir.EngineType.SP,
                    ),
                ):
                    emit_body()

    nc.compile()
    return nc


def _wlayout(w):
    # [HID, DC] -> [128, KT*DC]: partition-major k-tile layout, contiguous DMA
    return np.ascontiguousarray(
        w.reshape(KT, 128, DC).transpose(1, 0, 2).reshape(128, KT * DC)
    ).astype(BF16)


def stage_inputs(q, k, v, attn_bias, explored, Wq, bq, Wk, bk, Wv, bv):
    """Host-side sharding/layout staging. Returns in_maps for 8 cores."""
    scale = DK ** -0.5
    xT = {
        "q": np.ascontiguousarray(np.asarray(q, np.float32).T).astype(BF16),
        "k": np.ascontiguousarray(np.asarray(k, np.float32).T).astype(BF16),
        "v": np.ascontiguousarray(np.asarray(v, np.float32).T).astype(BF16),
    }
    Wq = np.asarray(Wq, np.float32) * scale
    bq = np.asarray(bq, np.float32) * scale
    Wk = np.asarray(Wk, np.float32)
    bk = np.asarray(bk, np.float32)
    Wv = np.asarray(Wv, np.float32)
    bv = np.asarray(bv, np.float32)
    attn_bias = np.asarray(attn_bias, np.float32)
    explored = np.asarray(explored)

    # keep[n, m]: True where the score survives; transposed views below
    keep = np.ones((N, N), dtype=bool)
    keep[1:, 1:] = explored != 0
    keepT = keep.T
    invmaskT = (~keepT).astype(F8)  # 1.0 at masked entries

    in_maps = []
    for c in range(NCORES):
        cols = slice(c * DC, (c + 1) * DC)
        h0 = HPC * c
        # bias^T per head with -256 folded in at masked entries: exp
        # underflows to 0 there; the correction matmul restores the +1s.
        bt = np.ascontiguousarray(attn_bias[h0 : h0 + HPC].transpose(0, 2, 1))
        bt = np.where(keepT[None], bt, np.float32(CFG["mask_neg"]))
        bt = bt.astype(F8).reshape(HPC * N, N)
        in_maps.append(
            {
                "xTq": xT["q"],
                "xTk": xT["k"],
                "xTv": xT["v"],
                "Wq": _wlayout(Wq[:, cols]),
                "Wk": _wlayout(Wk[:, cols]),
                "Wv": _wlayout(Wv[:, cols]),
                "bq": bq[cols].reshape(DC, 1).copy(),
                "bk": bk[cols].reshape(DC, 1).copy(),
                "bv": bv[cols].reshape(DC, 1).copy(),
                "biasT": bt,
                "invmaskT": invmaskT,
            }
        )
    return in_maps


def assemble_output(results):
    """results: list of 8 dicts with 'outD' [2048, 128] f16."""
    out = np.empty((N, HEADS * DK), dtype=np.float32)
    for c in range(NCORES):
        out[:, c * DC : (c + 1) * DC] = np.asarray(results[c]["outD"], np.float32)
    return out


def get_compiled(repeat=1):
    key = ("nc", repeat, tuple(sorted(CFG.items())))
    if key not in _cache:
        _cache[key] = _build(repeat)
    return _cache[key]


def kernel(**inputs) -> np.ndarray:
    from concourse.bass_utils import run_bass_kernel_spmd

    nc = get_compiled()
    in_maps = stage_inputs(**inputs)
    res = run_bass_kernel_spmd(nc, in_maps, core_ids=list(range(NCORES)))
    return assemble_output(res.results)
